# revision 1
# baseline (speedup 1.0000x reference)
"""Multi-head attention (RoPE + softmax) forward for Trainium2, 8 NeuronCores.

Problem: B=4, S=2048, D=2048, H=16 heads (hd=128), fp32 in/out.

Sharding: core c handles batch b = c//2 and head-group g = c%2 (8 heads).
Each core computes QKV projections for its batch restricted to its head
group, rotary, attention, and a partial output projection (contraction
over its 1024 features of wo).  The two partials per batch are summed on
the host.

Layout tricks (all host-side prep, free):
  - x is fed transposed (xT [D, S]) so it serves directly as matmul
    operands for both transposed (q/k) and natural (v) projections.
  - wq/wk columns are permuted per head so rotary pairs land in
    half-layout (real parts rows 0:64, imag rows 64:128 of each head
    block).  Rotary then works on contiguous partition halves.
  - mask folds into the softmax exp as a per-partition bias (scores are
    computed k-major: [k partitions, q free]).
  - softmax skips max-subtraction (inputs are well-scaled gaussians;
    scores are bounded ~|7|, exp stays in fp32 range).

All matmuls run as float32r (full PE rate at N=512, TF32-ish accuracy).
"""

import math

import numpy as np

B, S, D = 4, 2048, 2048
H_PER_CORE = 8  # heads per core
HD = 128  # head dim
F = 1024  # features per core (head group)
P = 128
DT = D // P  # 16 contraction tiles
NCORES = 8
SCALE = 1.0 / math.sqrt(HD)

_CACHE = {}


def _build():
    import concourse.bacc as bacc
    import concourse.mybir as mybir
    import concourse.tile as tile

    f32 = mybir.dt.float32
    f32r = mybir.dt.float32r
    EXP = mybir.ActivationFunctionType.Exp

    nc = bacc.Bacc("TRN2", target_bir_lowering=False, debug=False, num_devices=NCORES)

    xT = nc.dram_tensor("xT", [D, S], f32r, kind="ExternalInput")
    wq = nc.dram_tensor("wq", [D, F], f32r, kind="ExternalInput")
    wk = nc.dram_tensor("wk", [D, F], f32r, kind="ExternalInput")
    wv = nc.dram_tensor("wv", [D, F], f32r, kind="ExternalInput")
    wo = nc.dram_tensor("wo", [F, D], f32r, kind="ExternalInput")
    cosT_d = nc.dram_tensor("cosT", [P, S], f32, kind="ExternalInput")
    sinT_d = nc.dram_tensor("sinT", [P, S], f32, kind="ExternalInput")
    mask_d = nc.dram_tensor("maskT", [P, DT], f32, kind="ExternalInput")
    ones_k_d = nc.dram_tensor("ones_k", [P, 1], f32r, kind="ExternalInput")
    ones_p_d = nc.dram_tensor("ones_p", [1, P], f32r, kind="ExternalInput")
    out_d = nc.dram_tensor("out", [S, D], f32, kind="ExternalOutput")

    qT_d = nc.dram_tensor("qT_scratch", [F, S], f32r, kind="Internal")
    kT_d = nc.dram_tensor("kT_scratch", [F, S], f32r, kind="Internal")
    v_d = nc.dram_tensor("v_scratch", [S, F], f32r, kind="Internal")

    with tile.TileContext(nc) as tc, nc.allow_low_precision(
        reason="float32r tiles feeding fp32r matmuls; PSUM accumulation stays fp32"
    ):
        with tc.tile_pool(name="const", bufs=1) as constp:
            cos_sb = constp.tile([P, S], f32)
            sin_sb = constp.tile([P, S], f32)
            mask_sb = constp.tile([P, DT], f32)
            ones_k = constp.tile([P, 1], f32r)
            ones_p = constp.tile([1, P], f32r)
            nc.sync.dma_start(out=cos_sb[:], in_=cosT_d[:])
            nc.sync.dma_start(out=sin_sb[:], in_=sinT_d[:])
            nc.sync.dma_start(out=mask_sb[:], in_=mask_d[:])
            nc.sync.dma_start(out=ones_k[:], in_=ones_k_d[:])
            nc.sync.dma_start(out=ones_p[:], in_=ones_p_d[:])

            # ---- Stage 1: projections (q, k rotary-transposed; v natural) ----
            with (
                tc.tile_pool(name="wpool", bufs=1) as wpool,
                tc.tile_pool(name="xpool", bufs=2) as xpool,
                tc.tile_pool(name="evict", bufs=4) as epool,
                tc.tile_pool(name="ps1", bufs=6, space="PSUM") as ps1,
            ):
                first_pass = True
                for wdram, odram in ((wq, qT_d), (wk, kT_d)):
                    w_sb = wpool.tile([P, DT * F], f32r, tag="w")
                    x0_sb = None
                    if first_pass:
                        # interleave x(sc=0) and w DMAs per d-tile so the first
                        # matmul waits on ~1.3 MiB instead of 8 MiB of weights
                        x0_sb = xpool.tile([P, DT * 512], f32r, tag="x")
                        for dt in range(DT):
                            nc.sync.dma_start(
                                out=x0_sb[:, dt * 512 : (dt + 1) * 512],
                                in_=xT[dt * P : (dt + 1) * P, 0:512],
                            )
                            nc.sync.dma_start(
                                out=w_sb[:, dt * F : (dt + 1) * F],
                                in_=wdram[dt * P : (dt + 1) * P, :],
                            )
                        first_pass = False
                    else:
                        for dt in range(DT):
                            nc.sync.dma_start(
                                out=w_sb[:, dt * F : (dt + 1) * F],
                                in_=wdram[dt * P : (dt + 1) * P, :],
                            )
                    for sc in range(4):  # s-chunks of 512
                        if sc == 0 and x0_sb is not None:
                            x_sb = x0_sb
                        else:
                            x_sb = xpool.tile([P, DT * 512], f32r, tag="x")
                            for dt in range(DT):
                                nc.sync.dma_start(
                                    out=x_sb[:, dt * 512 : (dt + 1) * 512],
                                    in_=xT[dt * P : (dt + 1) * P, sc * 512 : (sc + 1) * 512],
                                )
                        for ft in range(8):  # feature tiles = heads
                            ps = ps1.tile([P, 512], f32, tag="ps")
                            for dt in range(DT):
                                nc.tensor.matmul(
                                    ps[:],
                                    lhsT=w_sb[:, dt * F + ft * P : dt * F + (ft + 1) * P],
                                    rhs=x_sb[:, dt * 512 : (dt + 1) * 512],
                                    start=(dt == 0),
                                    stop=(dt == DT - 1),
                                )
                            # rotary + eviction
                            o_sb = epool.tile([P, 512], f32r, tag="evq")
                            t2 = epool.tile([P, 512], f32, tag="t2")
                            cs = cos_sb[:, sc * 512 : (sc + 1) * 512]
                            sn = sin_sb[:, sc * 512 : (sc + 1) * 512]
                            nc.vector.tensor_mul(t2[0:64, :], ps[64:128, :], sn[0:64, :])
                            nc.vector.tensor_mul(t2[64:128, :], ps[0:64, :], sn[64:128, :])
                            nc.vector.tensor_mul(o_sb[:], ps[:], cs)
                            nc.vector.tensor_add(o_sb[:], o_sb[:], t2[:])
                            nc.sync.dma_start(
                                out=odram[ft * P : (ft + 1) * P, sc * 512 : (sc + 1) * 512],
                                in_=o_sb[:],
                            )

                # v pass: v[s, f] natural layout
                w_sb = wpool.tile([P, DT * F], f32r, tag="w")
                for dt in range(DT):
                    nc.sync.dma_start(
                        out=w_sb[:, dt * F : (dt + 1) * F],
                        in_=wv[dt * P : (dt + 1) * P, :],
                    )
                for st in range(16):  # s-tiles of 128
                    xv_sb = xpool.tile([P, DT * P], f32r, tag="xv")
                    for dt in range(DT):
                        nc.sync.dma_start(
                            out=xv_sb[:, dt * P : (dt + 1) * P],
                            in_=xT[dt * P : (dt + 1) * P, st * P : (st + 1) * P],
                        )
                    for fc in range(2):  # feature chunks of 512
                        ps = ps1.tile([P, 512], f32, tag="ps")
                        for dt in range(DT):
                            nc.tensor.matmul(
                                ps[:],
                                lhsT=xv_sb[:, dt * P : (dt + 1) * P],
                                rhs=w_sb[:, dt * F + fc * 512 : dt * F + (fc + 1) * 512],
                                start=(dt == 0),
                                stop=(dt == DT - 1),
                            )
                        v_sb = epool.tile([P, 512], f32r, tag="evv")
                        nc.scalar.copy(v_sb[:], ps[:])
                        nc.sync.dma_start(
                            out=v_d[st * P : (st + 1) * P, fc * 512 : (fc + 1) * 512],
                            in_=v_sb[:],
                        )

            # ---- Stage 2: attention per head ----
            with tc.tile_pool(name="attn", bufs=1) as apool:
                attn_sb = []
                for h in range(H_PER_CORE):
                    t = apool.tile([P, S], f32r, tag=f"attn{h}", name=f"attn{h}")
                    attn_sb.append(t)

                with (
                    tc.tile_pool(name="qkv2", bufs=2) as qkvp,
                    tc.tile_pool(name="exp2", bufs=4) as expp,
                    tc.tile_pool(name="small2", bufs=2) as smallp,
                    tc.tile_pool(name="ps_s", bufs=4, space="PSUM") as pss_pool,
                    tc.tile_pool(name="ps_o", bufs=2, space="PSUM") as pso_pool,
                    tc.tile_pool(name="ps_d", bufs=1, space="PSUM") as psd_pool,
                    tc.tile_pool(name="ps_b", bufs=1, space="PSUM") as psb_pool,
                ):
                    for h in range(H_PER_CORE):
                        q_sb = qkvp.tile([P, S], f32r, tag="q")
                        k_sb = qkvp.tile([P, S], f32r, tag="k")
                        v_sb = qkvp.tile([P, S], f32r, tag="v")
                        nc.sync.dma_start(out=q_sb[:], in_=qT_d[h * P : (h + 1) * P, :])
                        nc.sync.dma_start(out=k_sb[:], in_=kT_d[h * P : (h + 1) * P, :])
                        for kt in range(16):
                            nc.sync.dma_start(
                                out=v_sb[:, kt * P : (kt + 1) * P],
                                in_=v_d[kt * P : (kt + 1) * P, h * P : (h + 1) * P],
                            )
                        for qc in range(4):  # q chunks of 512
                            pso = pso_pool.tile([P, 512], f32, tag="pso")
                            psd = psd_pool.tile([P, 512], f32, tag="psd")
                            for kt in range(16):
                                pss = pss_pool.tile([P, 512], f32, tag="pss")
                                nc.tensor.matmul(
                                    pss[:],
                                    lhsT=k_sb[:, kt * P : (kt + 1) * P],
                                    rhs=q_sb[:, qc * 512 : (qc + 1) * 512],
                                    start=True,
                                    stop=True,
                                )
                                e_sb = expp.tile([P, 512], f32r, tag="e")
                                nc.scalar.activation(
                                    e_sb[:],
                                    pss[:],
                                    EXP,
                                    bias=mask_sb[:, kt : kt + 1],
                                    scale=SCALE,
                                )
                                nc.tensor.matmul(
                                    psd[0:1, :],
                                    lhsT=ones_k[:, 0:1],
                                    rhs=e_sb[:],
                                    start=(kt == 0),
                                    stop=(kt == 15),
                                )
                                nc.tensor.matmul(
                                    pso[:],
                                    lhsT=v_sb[:, kt * P : (kt + 1) * P],
                                    rhs=e_sb[:],
                                    start=(kt == 0),
                                    stop=(kt == 15),
                                )
                            r_sb = smallp.tile([1, 512], f32r, tag="r")
                            nc.vector.reciprocal(r_sb[:], psd[0:1, :])
                            psb = psb_pool.tile([P, 512], f32, tag="psb")
                            nc.tensor.matmul(
                                psb[:],
                                lhsT=ones_p[0:1, :],
                                rhs=r_sb[0:1, :],
                                start=True,
                                stop=True,
                            )
                            b_sb = smallp.tile([P, 512], f32, tag="b")
                            nc.scalar.copy(b_sb[:], psb[:])
                            nc.vector.tensor_mul(
                                attn_sb[h][:, qc * 512 : (qc + 1) * 512], pso[:], b_sb[:]
                            )

                # ---- Stage 3: output projection ----
                with (
                    tc.tile_pool(name="wo3", bufs=1) as wop,
                    tc.tile_pool(name="os3", bufs=4) as osp,
                    tc.tile_pool(name="ps3", bufs=4, space="PSUM") as ps3,
                ):
                    wo_sb = wop.tile([P, H_PER_CORE * D], f32r)
                    for h in range(H_PER_CORE):
                        nc.sync.dma_start(
                            out=wo_sb[:, h * D : (h + 1) * D],
                            in_=wo[h * P : (h + 1) * P, :],
                        )
                    for st in range(16):
                        for ec in range(4):
                            ps = ps3.tile([P, 512], f32, tag="ps3")
                            for h in range(H_PER_CORE):
                                nc.tensor.matmul(
                                    ps[:],
                                    lhsT=attn_sb[h][:, st * P : (st + 1) * P],
                                    rhs=wo_sb[:, h * D + ec * 512 : h * D + (ec + 1) * 512],
                                    start=(h == 0),
                                    stop=(h == H_PER_CORE - 1),
                                )
                            o_sb = osp.tile([P, 512], f32, tag="o3")
                            nc.scalar.copy(o_sb[:], ps[:])
                            nc.sync.dma_start(
                                out=out_d[st * P : (st + 1) * P, ec * 512 : (ec + 1) * 512],
                                in_=o_sb[:],
                            )

    nc.compile()
    return nc


def _host_prep(inputs):
    x = np.asarray(inputs["x"], np.float32)
    fc = np.asarray(inputs["freqs_cos"], np.float32)
    fs = np.asarray(inputs["freqs_sin"], np.float32)
    mask = np.asarray(inputs["mask"], np.float32)
    wq = np.asarray(inputs["wq"], np.float32)
    wk = np.asarray(inputs["wk"], np.float32)
    wv = np.asarray(inputs["wv"], np.float32)
    wo = np.asarray(inputs["wo"], np.float32)

    perm = np.concatenate([np.arange(0, HD, 2), np.arange(1, HD, 2)])
    cosT = np.ascontiguousarray(np.concatenate([fc.T, fc.T], 0))
    sinT = np.ascontiguousarray(np.concatenate([-fs.T, fs.T], 0))

    in_maps = []
    for c in range(NCORES):
        b, g = c // 2, c % 2
        colsel = np.concatenate([g * F + h * HD + perm for h in range(H_PER_CORE)])
        in_maps.append(
            {
                "xT": np.ascontiguousarray(x[b].T),
                "wq": np.ascontiguousarray(wq[:, colsel]),
                "wk": np.ascontiguousarray(wk[:, colsel]),
                "wv": np.ascontiguousarray(wv[:, g * F : (g + 1) * F]),
                "wo": np.ascontiguousarray(wo[g * F : (g + 1) * F, :]),
                "cosT": cosT,
                "sinT": sinT,
                "maskT": np.ascontiguousarray(mask[b].reshape(DT, P).T),
                "ones_k": np.ones((P, 1), np.float32),
                "ones_p": np.ones((1, P), np.float32),
            }
        )
    return in_maps


def kernel(**inputs):
    from concourse.bass_utils import run_bass_kernel_spmd

    if "nc" not in _CACHE:
        _CACHE["nc"] = _build()
    nc = _CACHE["nc"]

    in_maps = _host_prep(inputs)
    res = run_bass_kernel_spmd(nc, in_maps, core_ids=list(range(NCORES)))
    out = np.empty((B, S, D), np.float32)
    for b in range(B):
        out[b] = res.results[2 * b]["out"] + res.results[2 * b + 1]["out"]
    return out



# revision 3
# speedup vs baseline: 1.2820x; 1.2820x over previous
"""Multi-head attention (RoPE + softmax) forward for Trainium2, 8 NeuronCores.

v2: all matmul operands bf16 (fp32r measured ~6x slower per matmul on real HW),
q/k/v kept SBUF-resident (no DRAM scratch round trip), batched contiguous DMAs
via host-side weight relayout.

Problem: B=4, S=2048, D=2048, H=16 heads (hd=128), fp32 in/out.
Sharding: core c -> batch c//2, head-group c%2 (8 heads, 1024 features).
Partial outputs over the feature contraction are summed on the host.

Device layouts (host-prepped, all free):
  xh    [128, 16*2048] bf16 : xh[p, dt*2048+s] = x[b][s, dt*128+p]
  wqh   [8*128, 16*128] bf16: row ft*128+p, col dt*128+c -> wq[dt*128+p, colsel(ft,c)]
        (colsel applies the rotary half-layout permutation within each head)
  wkh   same as wqh for wk
  wvh   [2*128, 16*512] bf16: row fc*128+p, col dt*512+c -> wv[dt*128+p, fc*512+c]
  woh   [128, 8*2048] bf16  : woh[p, h*2048+c] = wo[h*128+p, c]
  cosT/sinT [128, 2048] bf16 rotary half-layout (rows 0:64 real, 64:128 imag)
  maskT [128, 16] f32: maskT[p, kt] = mask[b][kt*128+p]
"""

import math

import numpy as np

B, S, D = 4, 2048, 2048
H_PER_CORE = 8
HD = 128
F = 1024
P = 128
DT = D // P  # 16
NCORES = 8
SCALE = 1.0 / math.sqrt(HD)

_CACHE = {}


def _build():
    import concourse.bacc as bacc
    import concourse.mybir as mybir
    import concourse.tile as tile

    f32 = mybir.dt.float32
    bf16 = mybir.dt.bfloat16
    EXP = mybir.ActivationFunctionType.Exp

    nc = bacc.Bacc("TRN2", target_bir_lowering=False, debug=False, num_devices=NCORES)

    xh_d = nc.dram_tensor("xh", [P, DT * S], bf16, kind="ExternalInput")
    wq_d = nc.dram_tensor("wqh", [H_PER_CORE * P, DT * P], bf16, kind="ExternalInput")
    wk_d = nc.dram_tensor("wkh", [H_PER_CORE * P, DT * P], bf16, kind="ExternalInput")
    wv_d = nc.dram_tensor("wvh", [2 * P, DT * 512], bf16, kind="ExternalInput")
    wo_d = nc.dram_tensor("woh", [P, H_PER_CORE * S], bf16, kind="ExternalInput")
    cos_d = nc.dram_tensor("cosT", [P, S], bf16, kind="ExternalInput")
    sin_d = nc.dram_tensor("sinT", [P, S], bf16, kind="ExternalInput")
    mask_d = nc.dram_tensor("maskT", [P, DT], f32, kind="ExternalInput")
    ones_k_d = nc.dram_tensor("ones_k", [P, 1], bf16, kind="ExternalInput")
    ones_p_d = nc.dram_tensor("ones_p", [1, P], bf16, kind="ExternalInput")
    out_d = nc.dram_tensor("out", [S, D], f32, kind="ExternalOutput")

    with tile.TileContext(nc) as tc, nc.allow_low_precision(
        reason="bf16 operands feeding bf16 matmuls; PSUM accumulation stays fp32"
    ):
        with (
            tc.tile_pool(name="const", bufs=1) as constp,
            tc.tile_pool(name="qres", bufs=1) as qresp,
            tc.tile_pool(name="kres", bufs=1) as kresp,
            tc.tile_pool(name="vres", bufs=1) as vresp,
        ):
            cos_sb = constp.tile([P, S], bf16, name="cos_sb")
            sin_sb = constp.tile([P, S], bf16, name="sin_sb")
            mask_sb = constp.tile([P, DT], f32, name="mask_sb")
            ones_k = constp.tile([P, 1], bf16, name="ones_k_sb")
            ones_p = constp.tile([1, P], bf16, name="ones_p_sb")
            nc.sync.dma_start(out=cos_sb[:], in_=cos_d[:])
            nc.sync.dma_start(out=sin_sb[:], in_=sin_d[:])
            nc.sync.dma_start(out=mask_sb[:], in_=mask_d[:])
            nc.sync.dma_start(out=ones_k[:], in_=ones_k_d[:])
            nc.sync.dma_start(out=ones_p[:], in_=ones_p_d[:])

            qres = [qresp.tile([P, S], bf16, name=f"qres{h}") for h in range(H_PER_CORE)]
            kres = [kresp.tile([P, S], bf16, name=f"kres{h}") for h in range(H_PER_CORE)]
            vres = [vresp.tile([P, F], bf16, name=f"vres{st}") for st in range(DT)]

            # ---- Stage 1: projections; x and all of q/k/v stay in SBUF ----
            with (
                tc.tile_pool(name="xpool", bufs=1) as xpool,
                tc.tile_pool(name="wpool", bufs=2) as wpool,
                tc.tile_pool(name="wvpool", bufs=1) as wvpool,
                tc.tile_pool(name="rot", bufs=4) as rotp,
                tc.tile_pool(name="ps1", bufs=4, space="PSUM") as ps1,
            ):
                x_sb = xpool.tile([P, DT * S], bf16, name="x_sb")
                for dt in range(DT):
                    nc.sync.dma_start(
                        out=x_sb[:, dt * S : (dt + 1) * S],
                        in_=xh_d[:, dt * S : (dt + 1) * S],
                    )

                # q/k passes, per head: w block [128d, 16dt*128f]
                for wdram, res in ((wq_d, qres), (wk_d, kres)):
                    for ft in range(H_PER_CORE):
                        w_sb = wpool.tile([P, DT * P], bf16, tag="wqk", name=f"wqk{ft}")
                        nc.sync.dma_start(
                            out=w_sb[:], in_=wdram[ft * P : (ft + 1) * P, :]
                        )
                        for sc in range(4):
                            ps = ps1.tile([P, 512], f32, tag="ps", name=f"ps1_{ft}_{sc}")
                            for dt in range(DT):
                                nc.tensor.matmul(
                                    ps[:],
                                    lhsT=w_sb[:, dt * P : (dt + 1) * P],
                                    rhs=x_sb[:, dt * S + sc * 512 : dt * S + (sc + 1) * 512],
                                    start=(dt == 0),
                                    stop=(dt == DT - 1),
                                )
                            # rotary on DVE: bf16 copy then half-layout rotate
                            cs = cos_sb[:, sc * 512 : (sc + 1) * 512]
                            sn = sin_sb[:, sc * 512 : (sc + 1) * 512]
                            t2 = rotp.tile([P, 512], bf16, tag="t2", name="t2")
                            m1 = rotp.tile([P, 512], bf16, tag="m1", name="m1")
                            nc.vector.tensor_mul(t2[0:64, :], ps[64:128, :], sn[0:64, :])
                            nc.vector.tensor_mul(t2[64:128, :], ps[0:64, :], sn[64:128, :])
                            nc.vector.tensor_mul(m1[:], ps[:], cs)
                            nc.vector.tensor_add(
                                res[ft][:, sc * 512 : (sc + 1) * 512], m1[:], t2[:]
                            )

                # v pass: natural [s, f] layout, fc halves of 512
                for fc in range(2):
                    wv_sb = wvpool.tile([P, DT * 512], bf16, tag="wv", name=f"wv{fc}")
                    nc.sync.dma_start(out=wv_sb[:], in_=wv_d[fc * P : (fc + 1) * P, :])
                    for st in range(DT):
                        ps = ps1.tile([P, 512], f32, tag="ps", name=f"psv_{fc}_{st}")
                        for dt in range(DT):
                            nc.tensor.matmul(
                                ps[:],
                                lhsT=x_sb[:, dt * S + st * P : dt * S + (st + 1) * P],
                                rhs=wv_sb[:, dt * 512 : (dt + 1) * 512],
                                start=(dt == 0),
                                stop=(dt == DT - 1),
                            )
                        nc.scalar.copy(vres[st][:, fc * 512 : (fc + 1) * 512], ps[:])

            # ---- Stage 2: attention per head; stage 3 accumulates from SBUF ----
            with (
                tc.tile_pool(name="attn", bufs=1) as apool,
                tc.tile_pool(name="wo3", bufs=1) as wop,
            ):
                attn_sb = [
                    apool.tile([P, S], bf16, name=f"attn{h}") for h in range(H_PER_CORE)
                ]
                wo_sb = wop.tile([P, H_PER_CORE * S], bf16, name="wo_sb")
                for i in range(4):
                    nc.sync.dma_start(
                        out=wo_sb[:, i * 4096 : (i + 1) * 4096],
                        in_=wo_d[:, i * 4096 : (i + 1) * 4096],
                    )

                with (
                    tc.tile_pool(name="exp2", bufs=4) as expp,
                    tc.tile_pool(name="small2", bufs=3) as smallp,
                    tc.tile_pool(name="ps_s", bufs=3, space="PSUM") as pss_pool,
                    tc.tile_pool(name="ps_o", bufs=2, space="PSUM") as pso_pool,
                    tc.tile_pool(name="ps_d", bufs=2, space="PSUM") as psd_pool,
                    tc.tile_pool(name="ps_b", bufs=1, space="PSUM") as psb_pool,
                ):
                    for h, qc in [(h, qc) for h in range(H_PER_CORE) for qc in range(4)]:
                        pso = pso_pool.tile([P, 512], f32, tag="pso", name=f"pso{h}_{qc}")
                        psd = psd_pool.tile([1, 512], f32, tag="psd", name=f"psd{h}_{qc}")
                        for kt in range(DT):
                            pss = pss_pool.tile([P, 512], f32, tag="pss", name="pss")
                            nc.tensor.matmul(
                                pss[:],
                                lhsT=kres[h][:, kt * P : (kt + 1) * P],
                                rhs=qres[h][:, qc * 512 : (qc + 1) * 512],
                                start=True,
                                stop=True,
                            )
                            e_sb = expp.tile([P, 512], bf16, tag="e", name="e_sb")
                            nc.scalar.activation(
                                e_sb[:],
                                pss[:],
                                EXP,
                                bias=mask_sb[:, kt : kt + 1],
                                scale=SCALE,
                            )
                            nc.tensor.matmul(
                                psd[:],
                                lhsT=ones_k[:, 0:1],
                                rhs=e_sb[:],
                                start=(kt == 0),
                                stop=(kt == DT - 1),
                            )
                            nc.tensor.matmul(
                                pso[:],
                                lhsT=vres[kt][:, h * P : (h + 1) * P],
                                rhs=e_sb[:],
                                start=(kt == 0),
                                stop=(kt == DT - 1),
                            )
                        r_sb = smallp.tile([1, 512], bf16, tag="r", name="r_sb")
                        nc.vector.reciprocal(r_sb[:], psd[:])
                        psb = psb_pool.tile([P, 512], f32, tag="psb", name="psb")
                        nc.tensor.matmul(
                            psb[:],
                            lhsT=ones_p[0:1, :],
                            rhs=r_sb[0:1, :],
                            start=True,
                            stop=True,
                        )
                        b_sb = smallp.tile([P, 512], bf16, tag="b", name="b_sb")
                        nc.scalar.copy(b_sb[:], psb[:])
                        nc.vector.tensor_mul(
                            attn_sb[h][:, qc * 512 : (qc + 1) * 512], pso[:], b_sb[:]
                        )

                # ---- Stage 3: output projection ----
                with (
                    tc.tile_pool(name="os3", bufs=4) as osp,
                    tc.tile_pool(name="ps3", bufs=2, space="PSUM") as ps3,
                ):
                    for st in range(DT):
                        for ec in range(4):
                            ps = ps3.tile([P, 512], f32, tag="ps3", name="ps3t")
                            for h in range(H_PER_CORE):
                                nc.tensor.matmul(
                                    ps[:],
                                    lhsT=attn_sb[h][:, st * P : (st + 1) * P],
                                    rhs=wo_sb[:, h * S + ec * 512 : h * S + (ec + 1) * 512],
                                    start=(h == 0),
                                    stop=(h == H_PER_CORE - 1),
                                )
                            o_sb = osp.tile([P, 512], f32, tag="o3", name="o3")
                            nc.scalar.copy(o_sb[:], ps[:])
                            nc.sync.dma_start(
                                out=out_d[st * P : (st + 1) * P, ec * 512 : (ec + 1) * 512],
                                in_=o_sb[:],
                            )

    nc.compile()
    return nc


def _host_prep(inputs):
    import ml_dtypes

    bf = ml_dtypes.bfloat16
    x = np.asarray(inputs["x"], np.float32)
    fc = np.asarray(inputs["freqs_cos"], np.float32)
    fs = np.asarray(inputs["freqs_sin"], np.float32)
    mask = np.asarray(inputs["mask"], np.float32)
    wq = np.asarray(inputs["wq"], np.float32)
    wk = np.asarray(inputs["wk"], np.float32)
    wv = np.asarray(inputs["wv"], np.float32)
    wo = np.asarray(inputs["wo"], np.float32)

    perm = np.concatenate([np.arange(0, HD, 2), np.arange(1, HD, 2)])
    cosT = np.ascontiguousarray(np.concatenate([fc.T, fc.T], 0)).astype(bf)
    sinT = np.ascontiguousarray(np.concatenate([-fs.T, fs.T], 0)).astype(bf)

    in_maps = []
    for c in range(NCORES):
        b, g = c // 2, c % 2
        # xh[p, dt*S + s] = x[b][s, dt*128+p]
        xh = np.ascontiguousarray(
            x[b].T.reshape(DT, P, S).transpose(1, 0, 2).reshape(P, DT * S)
        ).astype(bf)
        # wqh[ft*128+p, dt*128+c] = wq[dt*128+p, g*F + ft*HD + perm[c]]
        def wqk_layout(w):
            cols = w[:, g * F : (g + 1) * F]  # [D, F]
            cols = cols.reshape(D, H_PER_CORE, HD)[:, :, perm]  # perm within head
            # -> [ft, p, dt, c]
            arr = cols.reshape(DT, P, H_PER_CORE, HD).transpose(2, 1, 0, 3)
            return np.ascontiguousarray(arr.reshape(H_PER_CORE * P, DT * P)).astype(bf)

        # wvh[fc*128+p, dt*512+c] = wv[dt*128+p, g*F + fc*512+c]
        vcols = wv[:, g * F : (g + 1) * F].reshape(DT, P, 2, 512).transpose(2, 1, 0, 3)
        wvh = np.ascontiguousarray(vcols.reshape(2 * P, DT * 512)).astype(bf)
        # woh[p, h*S + c] = wo[g*F + h*128 + p, c]
        woh = np.ascontiguousarray(
            wo[g * F : (g + 1) * F, :].reshape(H_PER_CORE, P, S).transpose(1, 0, 2).reshape(P, H_PER_CORE * S)
        ).astype(bf)
        in_maps.append(
            {
                "xh": xh,
                "wqh": wqk_layout(wq),
                "wkh": wqk_layout(wk),
                "wvh": wvh,
                "woh": woh,
                "cosT": cosT,
                "sinT": sinT,
                "maskT": np.ascontiguousarray(mask[b].reshape(DT, P).T),
                "ones_k": np.ones((P, 1), bf),
                "ones_p": np.ones((1, P), bf),
            }
        )
    return in_maps


def kernel(**inputs):
    from concourse.bass_utils import run_bass_kernel_spmd

    if "nc" not in _CACHE:
        _CACHE["nc"] = _build()
    nc = _CACHE["nc"]

    in_maps = _host_prep(inputs)
    res = run_bass_kernel_spmd(nc, in_maps, core_ids=list(range(NCORES)))
    out = np.empty((B, S, D), np.float32)
    for b in range(B):
        out[b] = res.results[2 * b]["out"] + res.results[2 * b + 1]["out"]
    return out


# revision 4
# speedup vs baseline: 1.5433x; 1.2038x over previous
"""Multi-head attention (RoPE + softmax) forward for Trainium2, 8 NeuronCores.

v4 over v3:
  - attention(h) interleaved with q-projection(h+1): Act(exp) and DVE work of
    attention hides under projection matmul work; PE stays continuously busy
    (p-state) and is the single roofline engine.
  - softmax rowsum off the PE: e tiles folded on DVE with exp(mask) weights
    (scalar_tensor_tensor, 16-deep bf16 fold), one ones-matmul per (h,qc) for
    the cross-partition fp32 sum, reciprocal on DVE, broadcast on gpsimd.
  - attn output streams to DRAM (frees SBUF for x residency), reloaded for
    stage 3 into the space x vacates.

All matmul operands bf16, PSUM fp32. Layouts: see v2/v3 docstrings.
"""

import math
from contextlib import ExitStack

import numpy as np

B, S, D = 4, 2048, 2048
H_PER_CORE = 8
HD = 128
F = 1024
P = 128
DT = D // P  # 16
NCORES = 8
SCALE = 1.0 / math.sqrt(HD)

_CACHE = {}


def _build():
    import concourse.bacc as bacc
    import concourse.mybir as mybir
    import concourse.tile as tile

    f32 = mybir.dt.float32
    bf16 = mybir.dt.bfloat16
    EXP = mybir.ActivationFunctionType.Exp
    MULT = mybir.AluOpType.mult
    ADD = mybir.AluOpType.add

    nc = bacc.Bacc("TRN2", target_bir_lowering=False, debug=False, num_devices=NCORES)

    xh_d = nc.dram_tensor("xh", [P, DT * S], bf16, kind="ExternalInput")
    wq_d = nc.dram_tensor("wqh", [H_PER_CORE * P, DT * P], bf16, kind="ExternalInput")
    wk_d = nc.dram_tensor("wkh", [H_PER_CORE * P, DT * P], bf16, kind="ExternalInput")
    wv_d = nc.dram_tensor("wvh", [2 * P, DT * 512], bf16, kind="ExternalInput")
    wo_d = nc.dram_tensor("woh", [P, H_PER_CORE * S], bf16, kind="ExternalInput")
    cos_d = nc.dram_tensor("cosT", [P, S], bf16, kind="ExternalInput")
    sin_d = nc.dram_tensor("sinT", [P, S], bf16, kind="ExternalInput")
    em_d = nc.dram_tensor("emaskT", [P, DT], bf16, kind="ExternalInput")
    emf_d = nc.dram_tensor("emaskTf", [P, DT], f32, kind="ExternalInput")
    ones_k_d = nc.dram_tensor("ones_k", [P, 1], bf16, kind="ExternalInput")
    out_d = nc.dram_tensor("out", [S, D], f32, kind="ExternalOutput")
    attn_d = nc.dram_tensor("attn_scratch", [H_PER_CORE * P, S], bf16, kind="Internal")

    with tile.TileContext(nc) as tc, nc.allow_low_precision(
        reason="bf16 operands feeding bf16 matmuls; PSUM accumulation stays fp32"
    ):
        with ExitStack() as outer:
            constp = outer.enter_context(tc.tile_pool(name="const", bufs=1))
            kresp = outer.enter_context(tc.tile_pool(name="kres", bufs=1))
            vresp = outer.enter_context(tc.tile_pool(name="vres", bufs=1))
            qringp = outer.enter_context(tc.tile_pool(name="qring", bufs=3))
            expp = outer.enter_context(tc.tile_pool(name="exp2", bufs=2))
            eaccp = outer.enter_context(tc.tile_pool(name="eacc", bufs=2))
            smallp = outer.enter_context(tc.tile_pool(name="small2", bufs=2))
            bcastp = outer.enter_context(tc.tile_pool(name="bcast", bufs=2))
            atilep = outer.enter_context(tc.tile_pool(name="atile", bufs=3))
            if True:
            em_sb = constp.tile([P, DT], bf16, name="em_sb")
            emf_sb = constp.tile([P, DT], f32, name="emf_sb")
            ones_k = constp.tile([P, 1], bf16, name="ones_k_sb")
            nc.sync.dma_start(out=em_sb[:], in_=em_d[:])
            nc.sync.dma_start(out=emf_sb[:], in_=emf_d[:])
            nc.sync.dma_start(out=ones_k[:], in_=ones_k_d[:])

            kres = [kresp.tile([P, S], bf16, name=f"kres{h}") for h in range(H_PER_CORE)]
            vres = [vresp.tile([P, F], bf16, name=f"vres{st}") for st in range(DT)]

            with ExitStack() as s1:
                cossinp = s1.enter_context(tc.tile_pool(name="cossin", bufs=1))
                xpool = s1.enter_context(tc.tile_pool(name="xpool", bufs=1))
                wpool = s1.enter_context(tc.tile_pool(name="wpool", bufs=2))
                rotp = s1.enter_context(tc.tile_pool(name="rot", bufs=4))
                ps1 = s1.enter_context(tc.tile_pool(name="ps1", bufs=2, space="PSUM"))
                pss_pool = s1.enter_context(tc.tile_pool(name="ps_s", bufs=2, space="PSUM"))
                pso_pool = s1.enter_context(tc.tile_pool(name="ps_o", bufs=1, space="PSUM"))
                psd_pool = s1.enter_context(tc.tile_pool(name="ps_d", bufs=1, space="PSUM"))
                cos_sb = cossinp.tile([P, S], bf16, name="cos_sb")
                sin_sb = cossinp.tile([P, S], bf16, name="sin_sb")
                nc.sync.dma_start(out=cos_sb[:], in_=cos_d[:])
                nc.sync.dma_start(out=sin_sb[:], in_=sin_d[:])

                x_sb = xpool.tile([P, DT * S], bf16, name="x_sb")
                for ch in range(32):
                    eng = nc.sync if ch % 2 == 0 else nc.scalar
                    eng.dma_start(
                        out=x_sb[:, ch * 1024 : (ch + 1) * 1024],
                        in_=xh_d[:, ch * 1024 : (ch + 1) * 1024],
                    )

                # ---- v pass (natural [s, f] layout, fc halves of 512) ----
                with tc.tile_pool(name="wvpool", bufs=1) as wvpool:
                    for fc in range(2):
                        wv_sb = wvpool.tile(
                            [P, DT * 512], bf16, tag="wv", name=f"wv{fc}"
                        )
                        nc.sync.dma_start(
                            out=wv_sb[:], in_=wv_d[fc * P : (fc + 1) * P, :]
                        )
                        for st in range(DT):
                            ps = ps1.tile([P, 512], f32, tag="ps", name="psv")
                            for dt in range(DT):
                                nc.tensor.matmul(
                                    ps[:],
                                    lhsT=x_sb[:, dt * S + st * P : dt * S + (st + 1) * P],
                                    rhs=wv_sb[:, dt * 512 : (dt + 1) * 512],
                                    start=(dt == 0),
                                    stop=(dt == DT - 1),
                                )
                            nc.scalar.copy(vres[st][:, fc * 512 : (fc + 1) * 512], ps[:])
                    for st in range(DT):
                        nc.vector.tensor_scalar_mul(
                            vres[st][:], vres[st][:], emf_sb[:, st : st + 1]
                        )

                def proj_head(wdram, ft, dest):
                    """dest [P, S] <- rotary(w_ft.T @ x)."""
                    w_sb = wpool.tile([P, DT * P], bf16, tag="wqk", name=f"w{ft}")
                    nc.sync.dma_start(out=w_sb[:], in_=wdram[ft * P : (ft + 1) * P, :])
                    for sc in range(4):
                        ps = ps1.tile([P, 512], f32, tag="ps", name="psp")
                        for dt in range(DT):
                            nc.tensor.matmul(
                                ps[:],
                                lhsT=w_sb[:, dt * P : (dt + 1) * P],
                                rhs=x_sb[:, dt * S + sc * 512 : dt * S + (sc + 1) * 512],
                                start=(dt == 0),
                                stop=(dt == DT - 1),
                            )
                        cs = cos_sb[:, sc * 512 : (sc + 1) * 512]
                        sn = sin_sb[:, sc * 512 : (sc + 1) * 512]
                        t2 = rotp.tile([P, 512], bf16, tag="t2", name="t2")
                        m1 = rotp.tile([P, 512], bf16, tag="m1", name="m1")
                        nc.vector.tensor_mul(t2[0:64, :], ps[64:128, :], sn[0:64, :])
                        nc.vector.tensor_mul(t2[64:128, :], ps[0:64, :], sn[64:128, :])
                        nc.vector.tensor_mul(m1[:], ps[:], cs)
                        nc.vector.tensor_add(
                            dest[:, sc * 512 : (sc + 1) * 512], m1[:], t2[:]
                        )

                # ---- k pass ----
                for ft in range(H_PER_CORE):
                    proj_head(wk_d, ft, kres[ft])

                # ---- q(0), then attention(h) interleaved with q(h+1) ----
                qtiles = {}
                qtiles[0] = qringp.tile([P, S], bf16, tag="q", name="q0")
                proj_head(wq_d, 0, qtiles[0])

                for h in range(H_PER_CORE):
                    q_sb = qtiles[h]
                    for qc in range(4):
                        pso = pso_pool.tile([P, 512], f32, tag="pso", name="pso")
                        e_acc = eaccp.tile([P, 512], bf16, tag="ea", name="e_acc")
                        for kp in range(DT // 2):
                            pss = pss_pool.tile([P, 1024], f32, tag="pss", name="pss")
                            for i in range(2):
                                kt = 2 * kp + i
                                nc.tensor.matmul(
                                    pss[:, i * 512 : (i + 1) * 512],
                                    lhsT=kres[h][:, kt * P : (kt + 1) * P],
                                    rhs=q_sb[:, qc * 512 : (qc + 1) * 512],
                                    start=True,
                                    stop=True,
                                )
                            e_sb = expp.tile([P, 1024], bf16, tag="e", name="e_sb")
                            nc.scalar.activation(e_sb[:], pss[:], EXP, scale=SCALE)
                            for i in range(2):
                                kt = 2 * kp + i
                                eh = e_sb[:, i * 512 : (i + 1) * 512]
                                if kt == 0:
                                    nc.vector.tensor_scalar_mul(
                                        e_acc[:], eh, emf_sb[:, 0:1]
                                    )
                                else:
                                    nc.vector.scalar_tensor_tensor(
                                        e_acc[:],
                                        eh,
                                        emf_sb[:, kt : kt + 1],
                                        e_acc[:],
                                        MULT,
                                        ADD,
                                    )
                                nc.tensor.matmul(
                                    pso[:],
                                    lhsT=vres[kt][:, h * P : (h + 1) * P],
                                    rhs=eh,
                                    start=(kt == 0),
                                    stop=(kt == DT - 1),
                                )
                        psd = psd_pool.tile([1, 512], f32, tag="psd", name="psd")
                        nc.tensor.matmul(
                            psd[:], lhsT=ones_k[:, 0:1], rhs=e_acc[:], start=True, stop=True
                        )
                        r_sb = smallp.tile([1, 512], f32, tag="r", name="r_sb")
                        nc.vector.reciprocal(r_sb[:], psd[:])
                        b_sb = bcastp.tile([P, 512], f32, tag="b", name="b_sb")
                        nc.gpsimd.partition_broadcast(b_sb[:], r_sb[0:1, :])
                        a_sb = atilep.tile([P, 512], bf16, tag="a", name="a_sb")
                        nc.vector.tensor_mul(a_sb[:], pso[:], b_sb[:])
                        nc.sync.dma_start(
                            out=attn_d[h * P : (h + 1) * P, qc * 512 : (qc + 1) * 512],
                            in_=a_sb[:],
                        )
                    if h + 1 < H_PER_CORE:
                        qtiles[h + 1] = qringp.tile([P, S], bf16, tag="q", name=f"q{h+1}")
                        proj_head(wq_d, h + 1, qtiles[h + 1])

            # ---- Stage 3: reload attn, output projection ----
            with ExitStack() as s3:
                aresp = s3.enter_context(tc.tile_pool(name="aresb", bufs=1))
                wop = s3.enter_context(tc.tile_pool(name="wo3", bufs=1))
                osp = s3.enter_context(tc.tile_pool(name="os3", bufs=6))
                ps3 = s3.enter_context(tc.tile_pool(name="ps3", bufs=3, space="PSUM"))
                wo_sb = wop.tile([P, H_PER_CORE * S], bf16, name="wo_sb")
                for i in range(4):
                    nc.scalar.dma_start(
                        out=wo_sb[:, i * 4096 : (i + 1) * 4096],
                        in_=wo_d[:, i * 4096 : (i + 1) * 4096],
                    )
                ares = [
                    aresp.tile([P, S], bf16, name=f"ares{h}") for h in range(H_PER_CORE)
                ]
                for h in range(H_PER_CORE):
                    nc.scalar.dma_start(
                        out=ares[h][:], in_=attn_d[h * P : (h + 1) * P, :]
                    )
                for st in range(DT):
                    for ec in range(4):
                        ps = ps3.tile([P, 512], f32, tag="ps3", name="ps3t")
                        for h in range(H_PER_CORE):
                            nc.tensor.matmul(
                                ps[:],
                                lhsT=ares[h][:, st * P : (st + 1) * P],
                                rhs=wo_sb[:, h * S + ec * 512 : h * S + (ec + 1) * 512],
                                start=(h == 0),
                                stop=(h == H_PER_CORE - 1),
                            )
                        o_sb = osp.tile([P, 512], f32, tag="o3", name="o3")
                        nc.scalar.copy(o_sb[:], ps[:])
                        nc.sync.dma_start(
                            out=out_d[st * P : (st + 1) * P, ec * 512 : (ec + 1) * 512],
                            in_=o_sb[:],
                        )

    nc.compile()
    return nc


def _host_prep(inputs):
    import ml_dtypes

    bf = ml_dtypes.bfloat16
    x = np.asarray(inputs["x"], np.float32)
    fc = np.asarray(inputs["freqs_cos"], np.float32)
    fs = np.asarray(inputs["freqs_sin"], np.float32)
    mask = np.asarray(inputs["mask"], np.float32)
    wq = np.asarray(inputs["wq"], np.float32)
    wk = np.asarray(inputs["wk"], np.float32)
    wv = np.asarray(inputs["wv"], np.float32)
    wo = np.asarray(inputs["wo"], np.float32)

    perm = np.concatenate([np.arange(0, HD, 2), np.arange(1, HD, 2)])
    cosT = np.ascontiguousarray(np.concatenate([fc.T, fc.T], 0)).astype(bf)
    sinT = np.ascontiguousarray(np.concatenate([-fs.T, fs.T], 0)).astype(bf)

    in_maps = []
    for c in range(NCORES):
        b, g = c // 2, c % 2
        xh = np.ascontiguousarray(
            x[b].T.reshape(DT, P, S).transpose(1, 0, 2).reshape(P, DT * S)
        ).astype(bf)

        def wqk_layout(w):
            cols = w[:, g * F : (g + 1) * F]
            cols = cols.reshape(D, H_PER_CORE, HD)[:, :, perm]
            arr = cols.reshape(DT, P, H_PER_CORE, HD).transpose(2, 1, 0, 3)
            return np.ascontiguousarray(arr.reshape(H_PER_CORE * P, DT * P)).astype(bf)

        vcols = wv[:, g * F : (g + 1) * F].reshape(DT, P, 2, 512).transpose(2, 1, 0, 3)
        wvh = np.ascontiguousarray(vcols.reshape(2 * P, DT * 512)).astype(bf)
        woh = np.ascontiguousarray(
            wo[g * F : (g + 1) * F, :].reshape(H_PER_CORE, P, S).transpose(1, 0, 2).reshape(P, H_PER_CORE * S)
        ).astype(bf)
        em = np.exp(mask[b]).reshape(DT, P).T
        in_maps.append(
            {
                "xh": xh,
                "wqh": wqk_layout(wq),
                "wkh": wqk_layout(wk),
                "wvh": wvh,
                "woh": woh,
                "cosT": cosT,
                "sinT": sinT,
                "emaskT": np.ascontiguousarray(em).astype(bf),
                "emaskTf": np.ascontiguousarray(em).astype(np.float32),
                "ones_k": np.ones((P, 1), bf),
            }
        )
    return in_maps


def kernel(**inputs):
    from concourse.bass_utils import run_bass_kernel_spmd

    if "nc" not in _CACHE:
        _CACHE["nc"] = _build()
    nc = _CACHE["nc"]

    in_maps = _host_prep(inputs)
    res = run_bass_kernel_spmd(nc, in_maps, core_ids=list(range(NCORES)))
    out = np.empty((B, S, D), np.float32)
    for b in range(B):
        out[b] = res.results[2 * b]["out"] + res.results[2 * b + 1]["out"]
    return out
